# revision 44
# baseline (speedup 1.0000x reference)
"""Trainium2 8-core tensor-parallel causal attention layer (prefill, pos=0).

Sharding: heads split across 8 cores (2 heads each). Per core:
  1. Q^T/K^T (head-dim-major) and V (token-major) projections for its 2 heads
     from a host-transposed bf16 copy of h,
  2. RoPE via an even/odd head-dim permutation baked into Wq/Wk columns,
  3. causal attention in the transposed domain (scores^T = K^T_tile.T @ Q^T;
     exp without max-subtraction — scores are O(1); row sums accumulated on
     DVE, reduced via a ones-vector matmul on a bf16 cast of the partial
     sums; per-group normalization runs with a one-group lag so the DVE
     FIFO never waits on the gpsimd broadcast),
  4. ONE AllGather per batch of the normalized attention outputs
     ([2*HD, S] bf16 staged in rank order -> [16*HD, S]); the last batch
     fires two half-token AllGathers instead so its output projection can
     start before the full batch finishes,
  5. a 256-row slice of the output d-dimension with its Wo column slice.
     Wo matmul blocks of batch b-1 are interleaved into the attention phase
     of batch b (its AllGather completed during proj(b)), filling TensorE
     idle slots.
Host-side: inputs transposed/sliced/cast bf16; outputs concatenated+transposed.
"""

import numpy as np
import ml_dtypes

import concourse.bass as bass
import concourse.tile as tile
from concourse import bacc, mybir
from concourse.bass_utils import run_bass_kernel_spmd

BF16 = mybir.dt.bfloat16
F32 = mybir.dt.float32
AF = mybir.ActivationFunctionType

B, S, D = 4, 2048, 2048
H, HD = 16, 128
NCORES = 8
HL = H // NCORES          # heads per core = 2
E = HL * HD               # per-core qkv width = 256
T = B * S                 # tokens = 8192
TT = 512                  # token tile (free dim)
NT_B = S // TT            # token tiles per batch = 4
DC = D // 128             # contraction chunks = 16
DS = D // NCORES          # output d-slice per core = 256
SCALE = 1.0 / np.sqrt(HD)

_cache = {}


def _build():
    nc = bacc.Bacc("TRN2", target_bir_lowering=False, debug=False,
                   num_devices=NCORES)

    hT_ext = nc.dram_tensor("hT", [D, T], BF16, kind="ExternalInput")
    wq_ext = nc.dram_tensor("wqT", [D, E], BF16, kind="ExternalInput")
    wk_ext = nc.dram_tensor("wkT", [D, E], BF16, kind="ExternalInput")
    wv_ext = nc.dram_tensor("wvT", [D, E], BF16, kind="ExternalInput")
    wo_ext = nc.dram_tensor("woT", [D, DS], BF16, kind="ExternalInput")
    # cos2 rows: [cos; cos]; sinpm rows: [sin; -sin] (for 2-mul RoPE)
    cos_ext = nc.dram_tensor("cos2T", [HD, S], BF16, kind="ExternalInput")
    sin_ext = nc.dram_tensor("sinpmT", [HD, S], BF16, kind="ExternalInput")
    mask_ext = nc.dram_tensor("maskT", [128, 4 * TT], BF16, kind="ExternalInput")
    out_ext = nc.dram_tensor("out", [DS, T], F32, kind="ExternalOutput")

    with tile.TileContext(nc) as tc:
        with (
            tc.tile_pool(name="weights", bufs=1) as wpool,
            tc.tile_pool(name="consts", bufs=1) as cpool,
            tc.tile_pool(name="ht", bufs=24) as htpool,
            tc.tile_pool(name="qkv", bufs=2) as qkvpool,
            tc.tile_pool(name="attn", bufs=10) as apool,
            tc.tile_pool(name="Spool", bufs=6) as Spool,
            tc.tile_pool(name="unpool", bufs=6) as unpool,
            tc.tile_pool(name="rtmp", bufs=2) as rpool,
            tc.tile_pool(name="small", bufs=4) as spool,
            tc.tile_pool(name="wor", bufs=30) as worpool,
            tc.tile_pool(name="ost", bufs=2) as ostpool,
            tc.tile_pool(name="ps", bufs=8, space="PSUM") as pspool,
            tc.tile_pool(name="dram", bufs=2, space="DRAM") as dpool,
        ):
            def load_w(ext, cols, tag):
                ts = []
                for dc in range(DC):
                    t = wpool.tile([128, cols], BF16, tag=f"{tag}{dc}",
                                   name=f"{tag}{dc}")
                    nc.gpsimd.dma_start(t[:], ext.ap()[dc * 128:(dc + 1) * 128, :])
                    ts.append(t)
                return ts

            wq_sb = load_w(wq_ext, E, "wq")
            wk_sb = load_w(wk_ext, E, "wk")

            cos_sb = cpool.tile([HD, S], BF16, tag="cos", name="cos")
            nc.scalar.dma_start(cos_sb[:], cos_ext.ap())
            sin_sb = cpool.tile([HD, S], BF16, tag="sin", name="sin")
            nc.scalar.dma_start(sin_sb[:], sin_ext.ap())
            mask_sb = cpool.tile([128, 4 * TT], BF16, tag="mask", name="mask")
            nc.scalar.dma_start(mask_sb[:], mask_ext.ap())
            ones_sb = cpool.tile([128, 1], BF16, tag="ones", name="ones")
            nc.vector.memset(ones_sb[:], 1.0)

            wv_sb = load_w(wv_ext, E, "wv")
            wo_sb = load_w(wo_ext, DS, "wo")

            # Warm-up AllGather: absorbs the CC-stream cold start (~25us)
            # during proj(0) instead of at batch 0's first real AllGather.
            wu_in = dpool.tile([128, 64], BF16, tag="wu_in", name="wu_in")
            nc.gpsimd.dma_start(wu_in[:], cos_ext.ap()[0:128, 0:64])
            wu_out = dpool.tile([NCORES * 128, 64], BF16, tag="wu_out",
                                name="wu_out", addr_space="Shared")
            nc.gpsimd.collective_compute(
                "AllGather", mybir.AluOpType.bypass,
                ins=[wu_in[:].opt()], outs=[wu_out[:].opt()],
                replica_groups=[list(range(NCORES))])

            def proj(b):
                """QKV projections + RoPE for batch b."""
                qT = [qkvpool.tile([HD, S], BF16, tag=f"qT{lh}",
                                   name=f"qT{lh}_{b}") for lh in range(HL)]
                kT = [qkvpool.tile([HD, S], BF16, tag=f"kT{lh}",
                                   name=f"kT{lh}_{b}") for lh in range(HL)]
                v_sb = [qkvpool.tile([128, E], BF16, tag=f"v{vt}",
                                     name=f"v{vt}_{b}")
                        for vt in range(S // 128)]
                for tt in range(NT_B):
                    gt = NT_B * b + tt
                    ht = []
                    for dc in range(DC):
                        t = htpool.tile([128, TT], BF16, tag="ht",
                                        name=f"ht{dc}_{gt}")
                        nc.sync.dma_start(
                            t[:], hT_ext.ap()[dc * 128:(dc + 1) * 128,
                                              gt * TT:(gt + 1) * TT])
                        ht.append(t)
                    cs = cos_sb[:, tt * TT:(tt + 1) * TT]
                    sn = sin_sb[:, tt * TT:(tt + 1) * TT]
                    for w_sb, dstT in ((wq_sb, qT), (wk_sb, kT)):
                        for lh in range(HL):
                            ps = pspool.tile([128, TT], F32, tag="ps",
                                             name=f"psp{b}_{tt}_{lh}")
                            for dc in range(DC):
                                nc.tensor.matmul(
                                    ps[:],
                                    lhsT=w_sb[dc][:, lh * HD:(lh + 1) * HD],
                                    rhs=ht[dc][:],
                                    start=(dc == 0), stop=(dc == DC - 1))
                            # RoPE: psum rows 0:64 = even pairs (x0), 64:128
                            # = odd (x1).  ACT casts psum->bf16 once, then
                            # 4 bf16 DVE ops at 2x rate:
                            #   A = pb * [cos; cos]
                            #   Bs[64:] = x0*sin ; Bs[:64] = x1*(-sin)
                            #   dst = A + Bs
                            dst = dstT[lh][:, tt * TT:(tt + 1) * TT]
                            pb = rpool.tile([128, TT], BF16, tag="pb",
                                            name=f"pb_{b}{tt}{lh}")
                            nc.scalar.copy(pb[:], ps[:])
                            ra = rpool.tile([128, TT], BF16, tag="ra",
                                            name=f"ra_{b}{tt}{lh}")
                            nc.vector.tensor_mul(ra[:], pb[:], cs)
                            rb = rpool.tile([128, TT], BF16, tag="rb",
                                            name=f"rb_{b}{tt}{lh}")
                            nc.vector.tensor_mul(rb[64:128, :], pb[0:64, :],
                                                 sn[0:64, :])
                            nc.vector.tensor_mul(rb[0:64, :], pb[64:128, :],
                                                 sn[64:128, :])
                            nc.vector.tensor_add(dst[:, :], ra[:], rb[:])
                    for vt in range(TT // 128):
                        ps = pspool.tile([128, E], F32, tag="ps",
                                         name=f"psv{b}_{tt}_{vt}")
                        for dc in range(DC):
                            nc.tensor.matmul(
                                ps[:],
                                lhsT=ht[dc][:, vt * 128:(vt + 1) * 128],
                                rhs=wv_sb[dc][:],
                                start=(dc == 0), stop=(dc == DC - 1))
                        nc.scalar.copy(v_sb[tt * 4 + vt][:], ps[:])
                return qT, kT, v_sb

            def norm_burst(burst, stage_target):
                """Normalize + stage a burst of (lh, qt) groups.  Phase-
                separated so each engine queue runs back-to-back with all
                cross-engine inputs already final: rowsums+recips first,
                then all broadcasts (gpsimd), then all muls (DVE), then all
                stage DMAs (gpsimd, right before the AllGather trigger)."""
                recips = []
                for b, lh, qt, Ssum, unnorm, _ in burst:
                    rps = pspool.tile([1, TT], F32, tag="ps",
                                      name=f"rs{b}_{lh}_{qt}")
                    nc.tensor.matmul(rps[:], lhsT=ones_sb[:, 0:1],
                                     rhs=Ssum[:], start=True, stop=True)
                    recip = spool.tile([1, TT], F32, tag="recip",
                                       name=f"rc{b}{lh}{qt}")
                    rscr = spool.tile([1, TT], F32, tag="rscr",
                                      name=f"rsc{b}{lh}{qt}")
                    nc.vector.reciprocal_approx_accurate(recip[:], rps[:],
                                                         rscr[:])
                    recips.append(recip)
                bcasts = []
                for (b, lh, qt, _, _, _), recip in zip(burst, recips):
                    bcast = spool.tile([128, TT], F32, tag="bcast",
                                       name=f"bc{b}{lh}{qt}")
                    nc.gpsimd.partition_broadcast(bcast[:], recip[:])
                    bcasts.append(bcast)
                agsts = []
                for (b, lh, qt, _, unnorm, _), bcast in zip(burst, bcasts):
                    agst = spool.tile([128, TT], BF16, tag="agst",
                                      name=f"ag{b}{lh}{qt}")
                    nc.vector.tensor_mul(agst[:], unnorm[:], bcast[:])
                    agsts.append(agst)
                for (b, lh, qt, _, _, _), agst in zip(burst, agsts):
                    dst, col = stage_target(lh, qt)
                    nc.gpsimd.dma_start(dst[:, col:col + TT], agst[:])

            def attn_groups(b, qT, kT, v_sb):
                """Yields after each (lh, qt) group.  Normalization for
                group i runs at the start of group i+1 (one-group lag).
                Per-lh AllGathers fire mid-attention (ag0 after lh0's
                groups, ag1 at the end); the last batch splits ag1 into
                two half-token AllGathers so wo(b) can start early."""
                split_tail = (b == B - 1)
                ag_ins, ag_outs = [], []
                for lh in range(HL):
                    if lh == 1 and split_tail:
                        # tail split: qt 0-2 in one AG, qt 3 alone so the
                        # last (latency-bound) AG is as small as possible
                        ag_ins.append([
                            dpool.tile([HD, 3 * TT], BF16, tag="ag_in1a",
                                       name=f"ag_in{b}_1a"),
                            dpool.tile([HD, TT], BF16, tag="ag_in1b",
                                       name=f"ag_in{b}_1b")])
                        ag_outs.append([
                            dpool.tile([NCORES * HD, 3 * TT], BF16,
                                       tag="ag_out1a", name=f"ag_out{b}_1a",
                                       addr_space="Shared"),
                            dpool.tile([NCORES * HD, TT], BF16,
                                       tag="ag_out1b", name=f"ag_out{b}_1b",
                                       addr_space="Shared")])
                    else:
                        ag_ins.append(dpool.tile(
                            [HD, S], BF16, tag=f"ag_in{lh}",
                            name=f"ag_in{b}_{lh}"))
                        ag_outs.append(dpool.tile(
                            [NCORES * HD, S], BF16, tag=f"ag_out{lh}",
                            name=f"ag_out{b}_{lh}", addr_space="Shared"))

                def stage_target(lh, qt):
                    dst = ag_ins[lh]
                    if isinstance(dst, list):
                        return (dst[0], qt * TT) if qt < 3 else \
                               (dst[1], 0)
                    return dst, qt * TT

                def fire_ag(lh, half=None):
                    qts = range(NT_B) if half is None else (
                        range(3) if half == 0 else range(3, NT_B))
                    burst = [r for r in records
                             if r[1] == lh and r[2] in qts]
                    records[:] = [r for r in records
                                  if not (r[1] == lh and r[2] in qts)]
                    norm_burst(burst, stage_target)
                    if half is None:
                        src_t, out_t = ag_ins[lh], ag_outs[lh]
                    else:
                        src_t, out_t = ag_ins[lh][half], ag_outs[lh][half]
                    nc.gpsimd.collective_compute(
                        "AllGather", mybir.AluOpType.bypass,
                        ins=[src_t[:].opt()],
                        outs=[out_t[:].opt()],
                        replica_groups=[list(range(NCORES))])

                records = []
                prev = None
                for lh in range(HL):
                    for qt in range(NT_B):
                        if prev == (0, NT_B - 1):
                            fire_ag(0)
                        if split_tail and prev == (1, 2):
                            fire_ag(1, half=0)
                        prev = (lh, qt)
                        n_kt = (qt + 1) * (TT // 128)
                        Ssum = Spool.tile([128, TT], BF16, tag="S",
                                          name=f"S{b}{lh}{qt}")
                        probs0 = None
                        aps = pspool.tile([128, TT], F32, tag="ps",
                                          name=f"aps{b}_{lh}_{qt}")

                        def attn_v(pp, pkt, stop):
                            nc.tensor.matmul(
                                aps[:],
                                lhsT=v_sb[pkt][:, lh * HD:(lh + 1) * HD],
                                rhs=pp[:],
                                start=(pkt == 0), stop=stop,
                                skip_group_check=True)

                        pend = []
                        for kt in range(n_kt):
                            sps = pspool.tile([128, TT], F32, tag="ps",
                                              name=f"sps{b}_{lh}_{qt}_{kt}")
                            nc.tensor.matmul(
                                sps[:],
                                lhsT=kT[lh][:, kt * 128:(kt + 1) * 128],
                                rhs=qT[lh][:, qt * TT:(qt + 1) * TT],
                                start=True, stop=True)
                            if len(pend) >= 2:
                                attn_v(*pend.pop(0), stop=False)
                            probs = apool.tile([128, TT], BF16, tag="probs",
                                               name=f"pr{b}_{lh}_{qt}_{kt}")
                            nc.scalar.activation(probs[:], sps[:], AF.Exp,
                                                 scale=float(SCALE))
                            diag = kt - qt * (TT // 128)
                            if diag >= 0:
                                nc.vector.tensor_mul(
                                    probs[:], probs[:],
                                    mask_sb[:, diag * TT:(diag + 1) * TT])
                            if kt == 0:
                                probs0 = probs
                            elif kt == 1:
                                nc.vector.tensor_add(Ssum[:], probs0[:],
                                                     probs[:])
                            else:
                                nc.vector.tensor_add(Ssum[:], Ssum[:], probs[:])
                            pend.append((probs, kt))
                        # yield BEFORE the AV tail: the interleaved wo
                        # block's matmuls fill the T-queue while the
                        # exp/mask/add chains catch up.
                        yield
                        while pend:
                            attn_v(*pend.pop(0), stop=(len(pend) == 0))

                        # free aps early: ACT copy psum -> sbuf bf16
                        unnorm = unpool.tile([128, TT], BF16, tag="unnorm",
                                             name=f"un{b}{lh}{qt}")
                        nc.scalar.copy(unnorm[:], aps[:])
                        records.append((b, lh, qt, Ssum, unnorm,
                                        stage_target))
                if split_tail:
                    fire_ag(1, half=1)
                else:
                    fire_ag(1)
                ag_slots[b] = ag_outs

            def wo_blocks(b):
                """8 yields: (st, m) output-projection blocks for batch b.
                ec 0-7 read ag_out[0] (even global heads), 8-15 ag_out[1]."""
                ag0, ag1 = ag_slots[b]
                for st in range(NT_B):
                    rts = []
                    for ec in range(DC):
                        if ec < 8:
                            source, col = ag0, st * TT
                        elif isinstance(ag1, list):
                            source, col = (ag1[0], st * TT) if st < 3 else \
                                          (ag1[1], 0)
                        else:
                            source, col = ag1, st * TT
                        r = (ec % 8) * 128
                        t = worpool.tile([128, TT], BF16, tag="wor",
                                         name=f"wor{ec}_{b}{st}")
                        nc.sync.dma_start(
                            t[:], source[r:r + 128, col:col + TT])
                        rts.append(t)
                    for m in range(DS // 128):
                        ps = pspool.tile([128, TT], F32, tag="ps",
                                         name=f"pso{b}_{st}_{m}")
                        for ec in range(DC):
                            nc.tensor.matmul(
                                ps[:],
                                lhsT=wo_sb[ec][:, m * 128:(m + 1) * 128],
                                rhs=rts[ec][:],
                                start=(ec == 0), stop=(ec == DC - 1))
                        ost = ostpool.tile([128, TT], F32, tag="ost",
                                           name=f"ost{b}{st}{m}")
                        nc.scalar.copy(ost[:], ps[:])
                        nc.scalar.dma_start(
                            out_ext.ap()[m * 128:(m + 1) * 128,
                                         b * S + st * TT:b * S + (st + 1) * TT],
                            ost[:])
                        yield

            ag_slots = {}
            wo_iter = None
            for b in range(B):
                q, k, v = proj(b)
                for _ in attn_groups(b, q, k, v):
                    if wo_iter is not None:
                        next(wo_iter, None)
                wo_iter = wo_blocks(b)
            for _ in wo_iter:  # tail: batch B-1's output projection
                pass

    nc.compile()
    return nc


def _prep_inputs(h, Wq, Wk, Wv, Wo, freqs_cos, freqs_sin):
    bf = ml_dtypes.bfloat16
    hT = np.ascontiguousarray(
        np.asarray(h, np.float32).transpose(2, 0, 1).reshape(D, T)).astype(bf)
    cosT = np.asarray(freqs_cos, np.float32).T           # [HD//2, S]
    sinT = np.asarray(freqs_sin, np.float32).T
    cos2T = np.ascontiguousarray(np.vstack([cosT, cosT])).astype(bf)
    sinpmT = np.ascontiguousarray(np.vstack([sinT, -sinT])).astype(bf)
    perm = np.concatenate([np.arange(0, HD, 2), np.arange(1, HD, 2)])
    p = np.arange(128)[:, None]
    j = np.arange(TT)[None, :]
    mask = np.concatenate(
        [(j >= 128 * i + p).astype(np.float32) for i in range(4)],
        axis=1).astype(bf)
    # wo e-rows permuted to AllGather order: even global heads then odd
    head_perm = np.concatenate([np.arange(0, H, 2), np.arange(1, H, 2)])

    Wq = np.asarray(Wq, np.float32); Wk = np.asarray(Wk, np.float32)
    Wv = np.asarray(Wv, np.float32); Wo = np.asarray(Wo, np.float32)
    in_maps = []
    for g in range(NCORES):
        rows = slice(E * g, E * (g + 1))
        wq_s = Wq[rows, :].reshape(HL, HD, D)[:, perm, :].reshape(E, D)
        wk_s = Wk[rows, :].reshape(HL, HD, D)[:, perm, :].reshape(E, D)
        wv_s = Wv[rows, :]
        woT = Wo[DS * g:DS * (g + 1), :].T          # [E_full, DS]
        woT = woT.reshape(H, HD, DS)[head_perm].reshape(H * HD, DS)
        in_maps.append({
            "hT": hT,
            "wqT": np.ascontiguousarray(wq_s.T).astype(bf),
            "wkT": np.ascontiguousarray(wk_s.T).astype(bf),
            "wvT": np.ascontiguousarray(wv_s.T).astype(bf),
            "woT": np.ascontiguousarray(woT).astype(bf),
            "cos2T": cos2T,
            "sinpmT": sinpmT,
            "maskT": np.ascontiguousarray(mask),
        })
    return in_maps


def _run(in_maps, **kw):
    if "nc" not in _cache:
        _cache["nc"] = _build()
    return run_bass_kernel_spmd(_cache["nc"], in_maps,
                                core_ids=list(range(NCORES)), **kw)


def kernel(h, Wq, Wk, Wv, Wo, K_cache=None, V_cache=None,
           freqs_cos=None, freqs_sin=None, pos=0, **_ignored):
    assert int(pos) == 0
    in_maps = _prep_inputs(h, Wq, Wk, Wv, Wo, freqs_cos, freqs_sin)
    res = _run(in_maps)
    fullT = np.concatenate(
        [np.asarray(res.results[g]["out"], np.float32) for g in range(NCORES)],
        axis=0)
    return np.ascontiguousarray(
        fullT.reshape(D, B, S).transpose(1, 2, 0)).astype(np.float32)


# revision 50
# speedup vs baseline: 1.0141x; 1.0141x over previous
"""Trainium2 8-core tensor-parallel causal attention layer (prefill, pos=0).

Sharding: heads split across 8 cores (2 heads each). Per core:
  1. Q^T/K^T (head-dim-major) and V (token-major) projections for its 2 heads
     from a host-transposed bf16 copy of h,
  2. RoPE via an even/odd head-dim permutation baked into Wq/Wk columns,
  3. causal attention in the transposed domain (scores^T = K^T_tile.T @ Q^T;
     exp without max-subtraction — scores are O(1); row sums accumulated on
     DVE, reduced via a ones-vector matmul on a bf16 cast of the partial
     sums; per-group normalization runs with a one-group lag so the DVE
     FIFO never waits on the gpsimd broadcast),
  4. ONE AllGather per batch of the normalized attention outputs
     ([2*HD, S] bf16 staged in rank order -> [16*HD, S]); the last batch
     fires two half-token AllGathers instead so its output projection can
     start before the full batch finishes,
  5. a 256-row slice of the output d-dimension with its Wo column slice.
     Wo matmul blocks of batch b-1 are interleaved into the attention phase
     of batch b (its AllGather completed during proj(b)), filling TensorE
     idle slots.
Host-side: inputs transposed/sliced/cast bf16; outputs concatenated+transposed.
"""

import numpy as np
import ml_dtypes

import concourse.bass as bass
import concourse.tile as tile
from concourse import bacc, mybir
from concourse.bass_utils import run_bass_kernel_spmd

BF16 = mybir.dt.bfloat16
F32 = mybir.dt.float32
AF = mybir.ActivationFunctionType

B, S, D = 4, 2048, 2048
H, HD = 16, 128
NCORES = 8
HL = H // NCORES          # heads per core = 2
E = HL * HD               # per-core qkv width = 256
T = B * S                 # tokens = 8192
TT = 512                  # token tile (free dim)
NT_B = S // TT            # token tiles per batch = 4
DC = D // 128             # contraction chunks = 16
DS = D // NCORES          # output d-slice per core = 256
SCALE = 1.0 / np.sqrt(HD)

_cache = {}


def _build():
    nc = bacc.Bacc("TRN2", target_bir_lowering=False, debug=False,
                   num_devices=NCORES)

    hT_ext = nc.dram_tensor("hT", [D, T], BF16, kind="ExternalInput")
    wq_ext = nc.dram_tensor("wqT", [D, E], BF16, kind="ExternalInput")
    wk_ext = nc.dram_tensor("wkT", [D, E], BF16, kind="ExternalInput")
    wv_ext = nc.dram_tensor("wvT", [D, E], BF16, kind="ExternalInput")
    wo_ext = nc.dram_tensor("woT", [D, DS], BF16, kind="ExternalInput")
    # cos2 rows: [cos; cos]; sinpm rows: [sin; -sin] (for 2-mul RoPE)
    cos_ext = nc.dram_tensor("cos2T", [HD, S], BF16, kind="ExternalInput")
    sin_ext = nc.dram_tensor("sinpmT", [HD, S], BF16, kind="ExternalInput")
    mask_ext = nc.dram_tensor("maskT", [128, 4 * TT], BF16, kind="ExternalInput")
    out_ext = nc.dram_tensor("out", [DS, T], F32, kind="ExternalOutput")

    with tile.TileContext(nc) as tc:
        with (
            tc.tile_pool(name="weights", bufs=1) as wpool,
            tc.tile_pool(name="consts", bufs=1) as cpool,
            tc.tile_pool(name="ht", bufs=24) as htpool,
            tc.tile_pool(name="qkv", bufs=2) as qkvpool,
            tc.tile_pool(name="attn", bufs=8) as apool,
            tc.tile_pool(name="Spool", bufs=6) as Spool,
            tc.tile_pool(name="unpool", bufs=6) as unpool,
            tc.tile_pool(name="rtmp", bufs=2) as rpool,
            tc.tile_pool(name="small", bufs=4) as spool,
            tc.tile_pool(name="wor", bufs=32) as worpool,
            tc.tile_pool(name="ost", bufs=2) as ostpool,
            tc.tile_pool(name="ps", bufs=8, space="PSUM") as pspool,
            tc.tile_pool(name="dram", bufs=2, space="DRAM") as dpool,
        ):
            def load_w(ext, cols, tag):
                ts = []
                for dc in range(DC):
                    t = wpool.tile([128, cols], BF16, tag=f"{tag}{dc}",
                                   name=f"{tag}{dc}")
                    nc.gpsimd.dma_start(t[:], ext.ap()[dc * 128:(dc + 1) * 128, :])
                    ts.append(t)
                return ts

            wq_sb = load_w(wq_ext, E, "wq")
            wk_sb = load_w(wk_ext, E, "wk")

            cos_sb = cpool.tile([HD, S], BF16, tag="cos", name="cos")
            nc.scalar.dma_start(cos_sb[:], cos_ext.ap())
            sin_sb = cpool.tile([HD, S], BF16, tag="sin", name="sin")
            nc.scalar.dma_start(sin_sb[:], sin_ext.ap())
            mask_sb = cpool.tile([128, 4 * TT], BF16, tag="mask", name="mask")
            nc.scalar.dma_start(mask_sb[:], mask_ext.ap())
            ones_sb = cpool.tile([128, 1], BF16, tag="ones", name="ones")
            nc.vector.memset(ones_sb[:], 1.0)

            wv_sb = load_w(wv_ext, E, "wv")
            wo_sb = load_w(wo_ext, DS, "wo")

            # Warm-up AllGather: absorbs the CC-stream cold start (~25us)
            # during proj(0) instead of at batch 0's first real AllGather.
            wu_in = dpool.tile([128, 64], BF16, tag="wu_in", name="wu_in")
            nc.gpsimd.dma_start(wu_in[:], cos_ext.ap()[0:128, 0:64])
            wu_out = dpool.tile([NCORES * 128, 64], BF16, tag="wu_out",
                                name="wu_out", addr_space="Shared")
            nc.gpsimd.collective_compute(
                "AllGather", mybir.AluOpType.bypass,
                ins=[wu_in[:].opt()], outs=[wu_out[:].opt()],
                replica_groups=[list(range(NCORES))])

            def proj(b):
                """QKV projections + RoPE for batch b."""
                qT = [qkvpool.tile([HD, S], BF16, tag=f"qT{lh}",
                                   name=f"qT{lh}_{b}") for lh in range(HL)]
                kT = [qkvpool.tile([HD, S], BF16, tag=f"kT{lh}",
                                   name=f"kT{lh}_{b}") for lh in range(HL)]
                v_sb = [qkvpool.tile([128, E], BF16, tag=f"v{vt}",
                                     name=f"v{vt}_{b}")
                        for vt in range(S // 128)]
                for tt in range(NT_B):
                    gt = NT_B * b + tt
                    ht = []
                    for dc in range(DC):
                        t = htpool.tile([128, TT], BF16, tag="ht",
                                        name=f"ht{dc}_{gt}")
                        nc.sync.dma_start(
                            t[:], hT_ext.ap()[dc * 128:(dc + 1) * 128,
                                              gt * TT:(gt + 1) * TT])
                        ht.append(t)
                    cs = cos_sb[:, tt * TT:(tt + 1) * TT]
                    sn = sin_sb[:, tt * TT:(tt + 1) * TT]
                    for w_sb, dstT in ((wq_sb, qT), (wk_sb, kT)):
                        for lh in range(HL):
                            ps = pspool.tile([128, TT], F32, tag="ps",
                                             name=f"psp{b}_{tt}_{lh}")
                            for dc in range(DC):
                                nc.tensor.matmul(
                                    ps[:],
                                    lhsT=w_sb[dc][:, lh * HD:(lh + 1) * HD],
                                    rhs=ht[dc][:],
                                    start=(dc == 0), stop=(dc == DC - 1))
                            # RoPE: psum rows 0:64 = even pairs (x0), 64:128
                            # = odd (x1).  ACT casts psum->bf16 once, then
                            # 4 bf16 DVE ops at 2x rate:
                            #   A = pb * [cos; cos]
                            #   Bs[64:] = x0*sin ; Bs[:64] = x1*(-sin)
                            #   dst = A + Bs
                            dst = dstT[lh][:, tt * TT:(tt + 1) * TT]
                            pb = rpool.tile([128, TT], BF16, tag="pb",
                                            name=f"pb_{b}{tt}{lh}")
                            nc.scalar.copy(pb[:], ps[:])
                            ra = rpool.tile([128, TT], BF16, tag="ra",
                                            name=f"ra_{b}{tt}{lh}")
                            nc.vector.tensor_mul(ra[:], pb[:], cs)
                            rb = rpool.tile([128, TT], BF16, tag="rb",
                                            name=f"rb_{b}{tt}{lh}")
                            nc.vector.tensor_mul(rb[64:128, :], pb[0:64, :],
                                                 sn[0:64, :])
                            nc.vector.tensor_mul(rb[0:64, :], pb[64:128, :],
                                                 sn[64:128, :])
                            nc.vector.tensor_add(dst[:, :], ra[:], rb[:])
                    for vt in range(TT // 128):
                        ps = pspool.tile([128, E], F32, tag="ps",
                                         name=f"psv{b}_{tt}_{vt}")
                        for dc in range(DC):
                            nc.tensor.matmul(
                                ps[:],
                                lhsT=ht[dc][:, vt * 128:(vt + 1) * 128],
                                rhs=wv_sb[dc][:],
                                start=(dc == 0), stop=(dc == DC - 1))
                        nc.scalar.copy(v_sb[tt * 4 + vt][:], ps[:])
                return qT, kT, v_sb

            def norm_burst(burst, stage_target):
                """Normalize + stage a burst of (lh, qt) groups.  Phase-
                separated so each engine queue runs back-to-back with all
                cross-engine inputs already final: rowsums+recips first,
                then all broadcasts (gpsimd), then all muls (DVE), then all
                stage DMAs (gpsimd, right before the AllGather trigger)."""
                recips = []
                for b, lh, qt, Ssum, unnorm, _ in burst:
                    rps = pspool.tile([1, TT], F32, tag="ps",
                                      name=f"rs{b}_{lh}_{qt}")
                    nc.tensor.matmul(rps[:], lhsT=ones_sb[:, 0:1],
                                     rhs=Ssum[:], start=True, stop=True)
                    recip = spool.tile([1, TT], F32, tag="recip",
                                       name=f"rc{b}{lh}{qt}")
                    rscr = spool.tile([1, TT], F32, tag="rscr",
                                      name=f"rsc{b}{lh}{qt}")
                    nc.vector.reciprocal_approx_accurate(recip[:], rps[:],
                                                         rscr[:])
                    recips.append(recip)
                bcasts = []
                for (b, lh, qt, _, _, _), recip in zip(burst, recips):
                    bcast = spool.tile([128, TT], F32, tag="bcast",
                                       name=f"bc{b}{lh}{qt}")
                    nc.gpsimd.partition_broadcast(bcast[:], recip[:])
                    bcasts.append(bcast)
                agsts = []
                for (b, lh, qt, _, unnorm, _), bcast in zip(burst, bcasts):
                    agst = spool.tile([128, TT], BF16, tag="agst",
                                      name=f"ag{b}{lh}{qt}")
                    nc.vector.tensor_mul(agst[:], unnorm[:], bcast[:])
                    agsts.append(agst)
                for (b, lh, qt, _, _, _), agst in zip(burst, agsts):
                    dst, col = stage_target(lh, qt)
                    nc.gpsimd.dma_start(dst[:, col:col + TT], agst[:])

            def attn_groups(b, qT, kT, v_sb):
                """Yields after each (lh, qt) group.  Normalization for
                group i runs at the start of group i+1 (one-group lag).
                Per-lh AllGathers fire mid-attention (ag0 after lh0's
                groups, ag1 at the end); the last batch splits ag1 into
                two half-token AllGathers so wo(b) can start early."""
                split_tail = (b == B - 1)
                ag_ins, ag_outs = [], []
                for lh in range(HL):
                    if lh == 1 and split_tail:
                        # tail split: qt 0-2 in one AG, qt 3 alone so the
                        # last (latency-bound) AG is as small as possible
                        ag_ins.append([
                            dpool.tile([HD, S // 2], BF16, tag="ag_in1a",
                                       name=f"ag_in{b}_1a"),
                            dpool.tile([HD, S // 2], BF16, tag="ag_in1b",
                                       name=f"ag_in{b}_1b")])
                        ag_outs.append([
                            dpool.tile([NCORES * HD, S // 2], BF16,
                                       tag="ag_out1a", name=f"ag_out{b}_1a",
                                       addr_space="Shared"),
                            dpool.tile([NCORES * HD, S // 2], BF16,
                                       tag="ag_out1b", name=f"ag_out{b}_1b",
                                       addr_space="Shared")])
                    else:
                        ag_ins.append(dpool.tile(
                            [HD, S], BF16, tag=f"ag_in{lh}",
                            name=f"ag_in{b}_{lh}"))
                        ag_outs.append(dpool.tile(
                            [NCORES * HD, S], BF16, tag=f"ag_out{lh}",
                            name=f"ag_out{b}_{lh}", addr_space="Shared"))

                def stage_target(lh, qt):
                    dst = ag_ins[lh]
                    if isinstance(dst, list):
                        return (dst[0], qt * TT) if qt < 2 else \
                               (dst[1], (qt - 2) * TT)
                    return dst, qt * TT

                def fire_ag(lh, half=None):
                    qts = range(NT_B) if half is None else (
                        range(2) if half == 0 else range(2, NT_B))
                    burst = [r for r in records
                             if r[1] == lh and r[2] in qts]
                    records[:] = [r for r in records
                                  if not (r[1] == lh and r[2] in qts)]
                    norm_burst(burst, stage_target)
                    if half is None:
                        src_t, out_t = ag_ins[lh], ag_outs[lh]
                    else:
                        src_t, out_t = ag_ins[lh][half], ag_outs[lh][half]
                    nc.gpsimd.collective_compute(
                        "AllGather", mybir.AluOpType.bypass,
                        ins=[src_t[:].opt()],
                        outs=[out_t[:].opt()],
                        replica_groups=[list(range(NCORES))])

                records = []
                prev = None
                for lh in range(HL):
                    for qt in range(NT_B):
                        if prev == (0, NT_B - 1):
                            fire_ag(0)
                        if split_tail and prev == (1, 1):
                            fire_ag(1, half=0)
                        prev = (lh, qt)
                        n_kt = (qt + 1) * (TT // 128)
                        Ssum = Spool.tile([128, TT], BF16, tag="S",
                                          name=f"S{b}{lh}{qt}")
                        probs0 = None
                        aps = pspool.tile([128, TT], F32, tag="ps",
                                          name=f"aps{b}_{lh}_{qt}")

                        def attn_v(pp, pkt, stop):
                            nc.tensor.matmul(
                                aps[:],
                                lhsT=v_sb[pkt][:, lh * HD:(lh + 1) * HD],
                                rhs=pp[:],
                                start=(pkt == 0), stop=stop,
                                skip_group_check=True)

                        pend = []
                        for kt in range(n_kt):
                            sps = pspool.tile([128, TT], F32, tag="ps",
                                              name=f"sps{b}_{lh}_{qt}_{kt}")
                            nc.tensor.matmul(
                                sps[:],
                                lhsT=kT[lh][:, kt * 128:(kt + 1) * 128],
                                rhs=qT[lh][:, qt * TT:(qt + 1) * TT],
                                start=True, stop=True)
                            if len(pend) >= 2:
                                attn_v(*pend.pop(0), stop=False)
                            probs = apool.tile([128, TT], BF16, tag="probs",
                                               name=f"pr{b}_{lh}_{qt}_{kt}")
                            nc.scalar.activation(probs[:], sps[:], AF.Exp,
                                                 scale=float(SCALE))
                            diag = kt - qt * (TT // 128)
                            if diag >= 0:
                                nc.vector.tensor_mul(
                                    probs[:], probs[:],
                                    mask_sb[:, diag * TT:(diag + 1) * TT])
                            if kt == 0:
                                probs0 = probs
                            elif kt == 1:
                                nc.vector.tensor_add(Ssum[:], probs0[:],
                                                     probs[:])
                            else:
                                nc.vector.tensor_add(Ssum[:], Ssum[:], probs[:])
                            pend.append((probs, kt))
                        # yield BEFORE the AV tail: the interleaved wo
                        # block's matmuls fill the T-queue while the
                        # exp/mask/add chains catch up.
                        yield
                        while pend:
                            attn_v(*pend.pop(0), stop=(len(pend) == 0))

                        # free aps early: ACT copy psum -> sbuf bf16
                        unnorm = unpool.tile([128, TT], BF16, tag="unnorm",
                                             name=f"un{b}{lh}{qt}")
                        nc.scalar.copy(unnorm[:], aps[:])
                        records.append((b, lh, qt, Ssum, unnorm,
                                        stage_target))
                if split_tail:
                    fire_ag(1, half=1)
                else:
                    fire_ag(1)
                ag_slots[b] = ag_outs

            def wo_blocks(b):
                """8 yields: (st, m) output-projection blocks for batch b.
                ec 0-7 read ag_out[0] (even global heads), 8-15 ag_out[1]."""
                ag0, ag1 = ag_slots[b]
                for st in range(NT_B):
                    rts = []
                    for ec in range(DC):
                        if ec < 8:
                            source, col = ag0, st * TT
                        elif isinstance(ag1, list):
                            source, col = (ag1[0], st * TT) if st < 2 else \
                                          (ag1[1], (st - 2) * TT)
                        else:
                            source, col = ag1, st * TT
                        r = (ec % 8) * 128
                        t = worpool.tile([128, TT], BF16, tag="wor",
                                         name=f"wor{ec}_{b}{st}")
                        nc.sync.dma_start(
                            t[:], source[r:r + 128, col:col + TT])
                        rts.append(t)
                    for m in range(DS // 128):
                        ps = pspool.tile([128, TT], F32, tag="ps",
                                         name=f"pso{b}_{st}_{m}")
                        for ec in range(DC):
                            nc.tensor.matmul(
                                ps[:],
                                lhsT=wo_sb[ec][:, m * 128:(m + 1) * 128],
                                rhs=rts[ec][:],
                                start=(ec == 0), stop=(ec == DC - 1))
                        ost = ostpool.tile([128, TT], F32, tag="ost",
                                           name=f"ost{b}{st}{m}")
                        nc.scalar.copy(ost[:], ps[:])
                        nc.scalar.dma_start(
                            out_ext.ap()[m * 128:(m + 1) * 128,
                                         b * S + st * TT:b * S + (st + 1) * TT],
                            ost[:])
                        yield

            ag_slots = {}
            wo_iter = None
            for b in range(B):
                q, k, v = proj(b)
                for _ in attn_groups(b, q, k, v):
                    if wo_iter is not None:
                        next(wo_iter, None)
                wo_iter = wo_blocks(b)
            for _ in wo_iter:  # tail: batch B-1's output projection
                pass

    nc.compile()
    return nc


def _prep_inputs(h, Wq, Wk, Wv, Wo, freqs_cos, freqs_sin):
    bf = ml_dtypes.bfloat16
    hT = np.ascontiguousarray(
        np.asarray(h, np.float32).transpose(2, 0, 1).reshape(D, T)).astype(bf)
    cosT = np.asarray(freqs_cos, np.float32).T           # [HD//2, S]
    sinT = np.asarray(freqs_sin, np.float32).T
    cos2T = np.ascontiguousarray(np.vstack([cosT, cosT])).astype(bf)
    sinpmT = np.ascontiguousarray(np.vstack([sinT, -sinT])).astype(bf)
    perm = np.concatenate([np.arange(0, HD, 2), np.arange(1, HD, 2)])
    p = np.arange(128)[:, None]
    j = np.arange(TT)[None, :]
    mask = np.concatenate(
        [(j >= 128 * i + p).astype(np.float32) for i in range(4)],
        axis=1).astype(bf)
    # wo e-rows permuted to AllGather order: even global heads then odd
    head_perm = np.concatenate([np.arange(0, H, 2), np.arange(1, H, 2)])

    Wq = np.asarray(Wq, np.float32); Wk = np.asarray(Wk, np.float32)
    Wv = np.asarray(Wv, np.float32); Wo = np.asarray(Wo, np.float32)
    in_maps = []
    for g in range(NCORES):
        rows = slice(E * g, E * (g + 1))
        wq_s = Wq[rows, :].reshape(HL, HD, D)[:, perm, :].reshape(E, D)
        wk_s = Wk[rows, :].reshape(HL, HD, D)[:, perm, :].reshape(E, D)
        wv_s = Wv[rows, :]
        woT = Wo[DS * g:DS * (g + 1), :].T          # [E_full, DS]
        woT = woT.reshape(H, HD, DS)[head_perm].reshape(H * HD, DS)
        in_maps.append({
            "hT": hT,
            "wqT": np.ascontiguousarray(wq_s.T).astype(bf),
            "wkT": np.ascontiguousarray(wk_s.T).astype(bf),
            "wvT": np.ascontiguousarray(wv_s.T).astype(bf),
            "woT": np.ascontiguousarray(woT).astype(bf),
            "cos2T": cos2T,
            "sinpmT": sinpmT,
            "maskT": np.ascontiguousarray(mask),
        })
    return in_maps


def _run(in_maps, **kw):
    if "nc" not in _cache:
        _cache["nc"] = _build()
    return run_bass_kernel_spmd(_cache["nc"], in_maps,
                                core_ids=list(range(NCORES)), **kw)


def kernel(h, Wq, Wk, Wv, Wo, K_cache=None, V_cache=None,
           freqs_cos=None, freqs_sin=None, pos=0, **_ignored):
    assert int(pos) == 0
    in_maps = _prep_inputs(h, Wq, Wk, Wv, Wo, freqs_cos, freqs_sin)
    res = _run(in_maps)
    fullT = np.concatenate(
        [np.asarray(res.results[g]["out"], np.float32) for g in range(NCORES)],
        axis=0)
    return np.ascontiguousarray(
        fullT.reshape(D, B, S).transpose(1, 2, 0)).astype(np.float32)
